# revision 1
# baseline (speedup 1.0000x reference)
"""Trainium2 Bass kernel for ActivatedAttention (B=4, T=2048, D=1024, f32).

  qkv = x @ W_in + b_in;  Q,K,V = split(qkv)
  Q = relu(rope(Q)); K = relu(rope(K)); V = relu(V)
  y = (Q @ K^T) @ V            # no softmax -> reassociate: y = Q @ (K^T @ V)
  out = swapaxes(group_norm(y), -2, -1).reshape(B, T, D)

Sharding: 8 cores = (batch b in 0..3) x (sequence half h in 0..1).
Each core projects its own 1024 rows, computes its partial K^T V, pair-
AllReduces it with its batch sibling (one 2MB bf16 collective; mesh latency
dominates, so chunking does not pay), then computes y = Q @ KtV and the
group norm for its rows.  The final swapaxes/reshape is pure data movement,
done on the host during unshard.

Tricks:
- RoPE channel permutation: W_in's Q/K columns are pre-permuted (even channels
  then odd channels) on the host so the interleaved-pair rotation becomes
  contiguous-block arithmetic on chip; the permutation cancels inside Q K^T.
- float32r matmuls (f32 with 11-bit mantissa, 4x faster than f32 on the PE);
  matmul operands are pre-rounded on the host or written as float32r on chip.
- biases land via a pre-broadcast tensor add on the PSUM-evacuation pass (K/V)
  or an ACT Identity-activation per-partition bias (Q, transposed layout).
- K/V/Q activations, the KtV exchange, and the y matmul are bf16.
- group norm: stats from PSUM, then ygn = y*rstd (DVE) - mean*rstd (GpSimd).
"""

from contextlib import ExitStack

import ml_dtypes
import numpy as np
import concourse.bass as bass
import concourse.tile as tile
from concourse import bacc, mybir, bass_utils

B, T, D = 4, 2048, 1024
TL = T // 2          # rows per core (sequence half)
EPS = 1e-5
THETA = 10000.0
NGROUP = 16          # groups per 512-column tile
GSIZE = 32

F32 = mybir.dt.float32
F32R = mybir.dt.float32r
BF16 = mybir.dt.bfloat16

KV_DT = BF16          # dtype of K/V activations + KtV collective
RG = [[0, 1], [2, 3], [4, 5], [6, 7]]
RELU = mybir.ActivationFunctionType.Relu
IDENT = mybir.ActivationFunctionType.Identity

_CACHE = {}


def _round_f32r(a):
    """Round-to-nearest-even f32 -> float32r (top 20 bits kept)."""
    b = np.ascontiguousarray(a).view(np.uint32).astype(np.uint64)
    bias_ = 0x7FF + ((b >> 12) & 1)
    b = (b + bias_) & ~np.uint64(0xFFF)
    return b.astype(np.uint32).view(np.float32)


def _build(gn_trivial):
    nc = bacc.Bacc("TRN2", target_bir_lowering=False, debug=False, num_devices=8)

    xT = nc.dram_tensor("xT", [D, TL], F32R, kind="ExternalInput")
    wq = nc.dram_tensor("wq", [D, D], F32R, kind="ExternalInput")
    wkv = nc.dram_tensor("wkv", [4 * D, 512], F32R, kind="ExternalInput")
    bq = nc.dram_tensor("bq", [128, 8], F32, kind="ExternalInput")
    bkvb = nc.dram_tensor("bkvb", [128, 2 * D], F32, kind="ExternalInput")
    cosq = nc.dram_tensor("cosq", [D // 2, TL], F32, kind="ExternalInput")
    sinq = nc.dram_tensor("sinq", [D // 2, TL], F32, kind="ExternalInput")
    cosk = nc.dram_tensor("cosk", [TL, D // 2], BF16, kind="ExternalInput")
    sink = nc.dram_tensor("sink", [TL, D // 2], BF16, kind="ExternalInput")
    if not gn_trivial:
        gnw = nc.dram_tensor("gnw", [128, D], F32, kind="ExternalInput")
        gnb = nc.dram_tensor("gnb", [128, D], F32, kind="ExternalInput")
    out = nc.dram_tensor("out", [2 * TL, 512], F32, kind="ExternalOutput")

    with tile.TileContext(nc) as tc, ExitStack() as st:
        psmall = st.enter_context(tc.tile_pool(name="small", bufs=1))
        pq = st.enter_context(tc.tile_pool(name="pq", bufs=1))
        pktv = st.enter_context(tc.tile_pool(name="pktv", bufs=1))
        pdram = st.enter_context(tc.tile_pool(name="pdram", bufs=2,
                                              space="DRAM"))

        bq_sb = psmall.tile([128, 8], F32, name="bq_sb")
        nc.sync.dma_start(bq_sb[:], bq[:])
        bkvb_sb = psmall.tile([128, 2 * D], F32, name="bkvb_sb")
        nc.sync.dma_start(bkvb_sb[:], bkvb[:])
        if not gn_trivial:
            gnw_sb = psmall.tile([128, D], F32, name="gnw_sb")
            nc.sync.dma_start(gnw_sb[:], gnw[:])
            gnb_sb = psmall.tile([128, D], F32, name="gnb_sb")
            nc.sync.dma_start(gnb_sb[:], gnb[:])
        eps_sb = psmall.tile([128, 1], F32, name="eps_sb")
        nc.vector.memset(eps_sb[:], EPS)

        qr = [pq.tile([128, TL], KV_DT, name=f"qr{j}", tag=f"qr{j}")
              for j in range(8)]
        ktv = [pktv.tile([128, D], KV_DT, name=f"ktv{c}", tag=f"ktv{c}")
               for c in range(8)]

        with tc.tile_pool(name="pxt", bufs=1) as pxt, \
             tc.tile_pool(name="ppsA", bufs=8, space="PSUM") as ppsA:
            xt = [pxt.tile([128, TL], F32R, name=f"xt{d}", tag=f"xt{d}")
                  for d in range(8)]

            # ================= phase A1: K'/V proj, K rope, KtV, collectives
            with tc.tile_pool(name="pkv", bufs=1) as pkv, \
                 tc.tile_pool(name="pwkv", bufs=12) as pwkv, \
                 tc.tile_pool(name="pkm", bufs=1) as pkm, \
                 tc.tile_pool(name="ptabk", bufs=3) as ptabk, \
                 tc.tile_pool(name="ptmpk", bufs=2) as ptmpk, \
                 tc.tile_pool(name="ppart", bufs=6) as ppart:

                kr = [pkv.tile([128, D], KV_DT, name=f"kr{t}", tag=f"kr{t}")
                      for t in range(8)]
                v_sb = [pkv.tile([128, D], KV_DT, name=f"v{t}", tag=f"v{t}")
                        for t in range(8)]
                km = [pkm.tile([128, D], KV_DT, name=f"km{t}", tag=f"km{t}")
                      for t in range(8)]

                def kv_proj(cb, tbs, wv):
                    for tb in tbs:
                        ps = ppsA.tile([128, 512], F32, name=f"pskv{cb}_{tb}",
                                       tag="psA")
                        for d in range(8):
                            nc.tensor.matmul(ps[:],
                                             xt[d][:, tb * 128:(tb + 1) * 128],
                                             wv[d][:], start=(d == 0),
                                             stop=(d == 7))
                        bias_s = bkvb_sb[:, cb * 512:(cb + 1) * 512]
                        if cb < 2:   # K' half -> km (+bias; rope next)
                            nc.vector.tensor_add(
                                km[tb][:, cb * 512:(cb + 1) * 512], ps[:],
                                bias_s)
                        else:        # V half -> +bias then relu in place
                            vslc = v_sb[tb][:, (cb - 2) * 512:(cb - 1) * 512]
                            nc.vector.tensor_add(vslc, ps[:], bias_s)
                            nc.scalar.activation(vslc, vslc, RELU)

                def load_wv(cb):
                    wv = [pwkv.tile([128, 512], F32R, name=f"wv{cb}_{d}",
                                    tag="wv") for d in range(8)]
                    for d in range(8):
                        if cb == 0:
                            nc.sync.dma_start(xt[d][:],
                                              xT[d * 128:(d + 1) * 128, :])
                        nc.sync.dma_start(
                            wv[d][:], wkv[cb * D + d * 128:
                                          cb * D + (d + 1) * 128, :])
                    return wv

                def ktv_half(half, tbs):
                    """Partial KtV over the given tb set + pair AllReduce.

                    cc buffers are th-major [2*D, 512]: rows dh*D + d1c*128
                    hold KtV[:, dh*512:(dh+1)*512], so the y phase's first
                    (th=0) operand half can be fetched before the second.
                    """
                    cc_in = pdram.tile([2 * D, 512], KV_DT, name=f"cci{half}",
                                       tag="cci")
                    cc_out = pdram.tile([2 * D, 512], KV_DT, name=f"cco{half}",
                                        tag="cco")
                    for d1c in range(8):
                        part = ppart.tile([128, D], KV_DT,
                                          name=f"part{half}_{d1c}", tag="part")
                        for dh in range(2):
                            ps = ppsA.tile([128, 512], F32,
                                           name=f"psk2_{half}_{d1c}_{dh}",
                                           tag="psA")
                            for i, tb in enumerate(tbs):
                                nc.tensor.matmul(
                                    ps[:],
                                    kr[tb][:, d1c * 128:(d1c + 1) * 128],
                                    v_sb[tb][:, dh * 512:(dh + 1) * 512],
                                    start=(i == 0), stop=(i == len(tbs) - 1))
                            dst = part[:, dh * 512:(dh + 1) * 512]
                            if dh == 0:
                                nc.vector.tensor_copy(dst, ps[:])
                            else:
                                nc.scalar.copy(dst, ps[:])
                            ro = dh * D + d1c * 128
                            nc.sync.dma_start(cc_in[ro:ro + 128, :], dst)
                    nc.gpsimd.collective_compute(
                        "AllReduce", mybir.AluOpType.add,
                        ins=[cc_in[:].opt()], outs=[cc_out[:].opt()],
                        replica_groups=RG)
                    return cc_out

                # PE warm-up during the initial DMA fill: junk matmuls on a
                # memset tile keep the HAM activity monitor at full clock
                warm = ptmpk.tile([128, 512], F32R, name="warm", tag="warm",
                                  bufs=1)
                nc.vector.memset(warm[:].bitcast(F32), 0.0)
                wps = ppsA.tile([128, 512], F32, name="wps", tag="psA")
                for i in range(40):
                    nc.tensor.matmul(wps[:], warm[:, 0:128], warm[:],
                                     start=(i == 0), stop=(i == 39))

                # K' projection (cols 0:1024 of [K'|V]) then rope per row-block
                wv01 = [load_wv(0), load_wv(1)]
                kv_proj(0, range(8), wv01[0])
                kv_proj(1, range(8), wv01[1])
                for tb in range(8):
                    ck = ptabk.tile([128, 512], BF16, name=f"ck{tb}", tag="ck")
                    sk = ptabk.tile([128, 512], BF16, name=f"sk{tb}", tag="sk")
                    nc.sync.dma_start(ck[:], cosk[tb * 128:(tb + 1) * 128, :])
                    nc.sync.dma_start(sk[:], sink[tb * 128:(tb + 1) * 128, :])
                    x1 = km[tb][:, 0:512]
                    x2 = km[tb][:, 512:1024]
                    t1 = ptmpk.tile([128, 512], KV_DT, name=f"t1k{tb}",
                                    tag="t1")
                    t2 = ptmpk.tile([128, 512], KV_DT, name=f"t2k{tb}",
                                    tag="t2")
                    nc.vector.tensor_mul(t1[:], x1, sk[:])       # x1*sin
                    nc.vector.tensor_mul(x1, x1, ck[:])          # x1 = x1*cos
                    nc.vector.tensor_mul(t2[:], x2, sk[:])       # x2*sin
                    nc.vector.tensor_sub(x1, x1, t2[:])          # r1
                    nc.vector.tensor_mul(x2, x2, ck[:])          # x2 = x2*cos
                    nc.vector.tensor_add(x2, x2, t1[:])          # r2
                    nc.scalar.activation(kr[tb][:, 0:512], x1, RELU)
                    nc.scalar.activation(kr[tb][:, 512:1024], x2, RELU)

                # V projection, then the full KtV partial + one collective
                kv_proj(2, range(8), load_wv(2))
                kv_proj(3, range(8), load_wv(3))
                cc_out_a = ktv_half(0, list(range(8)))

            # ================= phase A2: Q' proj (transposed layout) + rope
            with tc.tile_pool(name="pwq", bufs=1) as pwq, \
                 tc.tile_pool(name="pqm", bufs=1) as pqm, \
                 tc.tile_pool(name="ptabq", bufs=2) as ptabq, \
                 tc.tile_pool(name="ptmpq", bufs=2) as ptmpq:

                wq_sb = [pwq.tile([128, D], F32R, name=f"wq{d}", tag=f"wq{d}")
                         for d in range(8)]
                for d in range(8):
                    nc.sync.dma_start(wq_sb[d][:],
                                      wq[d * 128:(d + 1) * 128, :])
                qm = [pqm.tile([128, TL], F32, name=f"qm{j}", tag=f"qm{j}")
                      for j in range(8)]

                def q_proj(cp):
                    for th in range(2):
                        ps = ppsA.tile([128, 512], F32, name=f"psq{cp}_{th}",
                                       tag="psA")
                        for d in range(8):
                            nc.tensor.matmul(
                                ps[:], wq_sb[d][:, cp * 128:(cp + 1) * 128],
                                xt[d][:, th * 512:(th + 1) * 512],
                                start=(d == 0), stop=(d == 7))
                        nc.scalar.activation(
                            qm[cp][:, th * 512:(th + 1) * 512], ps[:],
                            IDENT, bias=bq_sb[:, cp:cp + 1])

                # pair (j, j+4) projected together so rope(j) starts while the
                # next pair is still on the PE
                for j in range(4):
                    q_proj(j)
                    q_proj(j + 4)
                    cq = ptabq.tile([128, TL], F32, name=f"cq{j}", tag="cq")
                    sq = ptabq.tile([128, TL], F32, name=f"sq{j}", tag="sq")
                    nc.sync.dma_start(cq[:], cosq[j * 128:(j + 1) * 128, :])
                    nc.sync.dma_start(sq[:], sinq[j * 128:(j + 1) * 128, :])
                    x1 = qm[j][:]
                    x2 = qm[j + 4][:]
                    t1 = ptmpq.tile([128, TL], F32, name=f"t1q{j}", tag="t1",
                                    bufs=1)
                    t2 = ptmpq.tile([128, TL], F32, name=f"t2q{j}", tag="t2",
                                    bufs=1)
                    nc.gpsimd.tensor_mul(t1[:], x1, sq[:])       # x1*sin
                    nc.vector.tensor_mul(x1, x1, cq[:])          # x1*cos
                    nc.vector.tensor_mul(t2[:], x2, sq[:])       # x2*sin
                    nc.vector.tensor_sub(x1, x1, t2[:])          # r1
                    nc.vector.tensor_mul(x2, x2, cq[:])          # x2*cos
                    nc.vector.tensor_add(x2, x2, t1[:])          # r2
                    nc.scalar.activation(qr[j][:], x1, RELU)
                    nc.scalar.activation(qr[j + 4][:], x2, RELU)

                # fetch the reduced KtV straight into the y operand
                # (bf16); th=0 halves first so the first y pass starts sooner
                for dh in range(2):
                    for d1c in range(8):
                        ro = dh * D + d1c * 128
                        nc.sync.dma_start(
                            ktv[d1c][:, dh * 512:(dh + 1) * 512],
                            cc_out_a[ro:ro + 128, :])

        # ================= phase C: y = Q' @ KtV + fused group norm ========
        with tc.tile_pool(name="pgn", bufs=4) as pgn, \
             tc.tile_pool(name="pstat", bufs=4) as pstat, \
             tc.tile_pool(name="ppsY", bufs=1, space="PSUM") as ppsY:
            inv32 = 1.0 / GSIZE
            for th in range(2):
                for tb in range(8):
                    ps = ppsY.tile([128, 512], F32, name=f"psy{th}_{tb}",
                                   tag=f"psy{tb % 4}")
                    for c in range(8):
                        nc.tensor.matmul(
                            ps[:], qr[c][:, tb * 128:(tb + 1) * 128],
                            ktv[c][:, th * 512:(th + 1) * 512],
                            start=(c == 0), stop=(c == 7))
                    ps3 = ps[:].rearrange("p (g c) -> p g c", g=NGROUP)
                    sums = pstat.tile([128, 16], F32, name=f"su{th}_{tb}",
                                      tag="su")
                    nc.vector.reduce_sum(sums[:], ps3,
                                         axis=mybir.AxisListType.X)
                    sqt = pgn.tile([128, 512], F32, name=f"sqt{th}_{tb}",
                                   tag="sqt")
                    nc.scalar.square(sqt[:], ps[:])
                    sums2 = pstat.tile([128, 16], F32, name=f"s2{th}_{tb}",
                                       tag="s2")
                    nc.vector.reduce_sum(
                        sums2[:],
                        sqt[:].rearrange("p (g c) -> p g c", g=NGROUP),
                        axis=mybir.AxisListType.X)
                    mean = pstat.tile([128, 16], F32, name=f"mn{th}_{tb}",
                                      tag="mn")
                    nc.vector.tensor_scalar_mul(mean[:], sums[:], inv32)
                    ex2 = pstat.tile([128, 16], F32, name=f"e2{th}_{tb}",
                                     tag="e2")
                    nc.gpsimd.tensor_scalar_mul(ex2[:], sums2[:], inv32)
                    msq = pstat.tile([128, 16], F32, name=f"mq{th}_{tb}",
                                     tag="mq")
                    nc.vector.tensor_mul(msq[:], mean[:], mean[:])
                    var = pstat.tile([128, 16], F32, name=f"va{th}_{tb}",
                                     tag="va")
                    nc.vector.tensor_sub(var[:], ex2[:], msq[:])
                    sd = pstat.tile([128, 16], F32, name=f"sd{th}_{tb}",
                                    tag="sd")
                    nc.scalar.activation(sd[:], var[:],
                                         mybir.ActivationFunctionType.Sqrt,
                                         bias=eps_sb[:])
                    rstd = pstat.tile([128, 16], F32, name=f"rs{th}_{tb}",
                                      tag="rs")
                    nc.vector.reciprocal(rstd[:], sd[:])
                    mr = pstat.tile([128, 16], F32, name=f"mr{th}_{tb}",
                                    tag="mr")
                    nc.vector.tensor_mul(mr[:], mean[:], rstd[:])

                    ygn = pgn.tile([128, 512], F32, name=f"yg{th}_{tb}",
                                   tag="ygn")
                    y3 = ygn[:].rearrange("p (g c) -> p g c", g=NGROUP)
                    nc.vector.tensor_mul(
                        y3, ps3, rstd[:].broadcast_to([128, NGROUP, GSIZE]))
                    nc.gpsimd.tensor_sub(
                        y3, y3, mr[:].broadcast_to([128, NGROUP, GSIZE]))
                    cs = slice(th * 512, (th + 1) * 512)
                    if not gn_trivial:
                        nc.gpsimd.tensor_mul(ygn[:], ygn[:], gnw_sb[:, cs])
                        nc.gpsimd.tensor_add(ygn[:], ygn[:], gnb_sb[:, cs])
                    ro = th * TL + tb * 128
                    nc.sync.dma_start(out[ro:ro + 128, :], ygn[:])

    nc.compile()
    return nc


def _get_nc(gn_trivial):
    key = ("nc", gn_trivial)
    if key not in _CACHE:
        _CACHE[key] = _build(gn_trivial)
    return _CACHE[key]


def _make_in_maps(x, W_in, b_in, gn_weight, gn_bias, gn_trivial):
    perm = np.concatenate([np.arange(0, D, 2), np.arange(1, D, 2)])
    wq_h = _round_f32r(np.ascontiguousarray(W_in[:, 0:D][:, perm]))
    wk = W_in[:, D:2 * D][:, perm]
    wv = W_in[:, 2 * D:3 * D]
    wkv_cat = np.concatenate([wk, wv], axis=1)          # [D, 2D]
    wkv_h = _round_f32r(np.ascontiguousarray(
        np.concatenate([wkv_cat[:, cb * 512:(cb + 1) * 512]
                        for cb in range(4)], axis=0)))    # [4D, 512]
    bq_h = np.ascontiguousarray(b_in[0:D][perm].reshape(8, 128).T)
    bkv_row = np.concatenate([b_in[D:2 * D][perm], b_in[2 * D:3 * D]])
    bkvb_h = np.ascontiguousarray(
        np.broadcast_to(bkv_row[None, :], (128, 2 * D))).astype(np.float32)
    gnw_h = np.ascontiguousarray(
        np.broadcast_to(np.float32(gn_weight)[None, :], (128, D)))
    gnb_h = np.ascontiguousarray(
        np.broadcast_to(np.float32(gn_bias)[None, :], (128, D)))

    inv_freq = (1.0 / (THETA ** (np.arange(0, D, 2, dtype=np.float32) / D))
                ).astype(np.float32)

    in_maps = []
    for core in range(8):
        b, h = divmod(core, 2)
        ts = np.arange(h * TL, (h + 1) * TL, dtype=np.float32)
        freqs = ts[:, None] * inv_freq[None, :]      # [TL, 512]
        cos_n = np.cos(freqs).astype(np.float32)
        sin_n = np.sin(freqs).astype(np.float32)
        xT_h = _round_f32r(
            np.ascontiguousarray(x[b, h * TL:(h + 1) * TL, :].T))
        m = {
            "xT": xT_h, "wq": wq_h, "wkv": wkv_h, "bq": bq_h, "bkvb": bkvb_h,
            "cosq": np.ascontiguousarray(cos_n.T),
            "sinq": np.ascontiguousarray(sin_n.T),
            "cosk": cos_n.astype(ml_dtypes.bfloat16),
            "sink": sin_n.astype(ml_dtypes.bfloat16),
        }
        if not gn_trivial:
            m["gnw"] = gnw_h
            m["gnb"] = gnb_h
        in_maps.append(m)
    return in_maps


def kernel(x, W_in, b_in, gn_weight, gn_bias, _trace=False):
    x = np.asarray(x, dtype=np.float32)
    W_in = np.asarray(W_in, dtype=np.float32)
    b_in = np.asarray(b_in, dtype=np.float32)
    gn_weight = np.asarray(gn_weight, dtype=np.float32)
    gn_bias = np.asarray(gn_bias, dtype=np.float32)

    gn_trivial = bool(np.all(gn_weight == 1.0) and np.all(gn_bias == 0.0))
    nc = _get_nc(gn_trivial)
    in_maps = _make_in_maps(x, W_in, b_in, gn_weight, gn_bias, gn_trivial)
    res = bass_utils.run_bass_kernel_spmd(nc, in_maps, core_ids=list(range(8)),
                                          trace=_trace)
    _CACHE["last_result"] = res

    outs = [np.concatenate([res.results[i]["out"][:TL],
                            res.results[i]["out"][TL:]], axis=1)
            for i in range(8)]                            # [TL, D] each
    full = np.empty((B, T, D), dtype=np.float32)
    for b in range(B):
        y_gn = np.concatenate([outs[2 * b], outs[2 * b + 1]], axis=0)  # [T,D]
        full[b] = y_gn.T.reshape(T, D)
    return full



# revision 2
# speedup vs baseline: 1.1540x; 1.1540x over previous
"""Trainium2 Bass kernel for ActivatedAttention (B=4, T=2048, D=1024, f32).

  qkv = x @ W_in + b_in;  Q,K,V = split(qkv)
  Q = relu(rope(Q)); K = relu(rope(K)); V = relu(V)
  y = (Q @ K^T) @ V            # no softmax -> reassociate: y = Q @ (K^T @ V)
  out = swapaxes(group_norm(y), -2, -1).reshape(B, T, D)

Sharding: 8 cores = (batch b in 0..3) x (sequence half h in 0..1).
Each core projects its own 1024 rows, computes its partial K^T V, pair-
AllReduces it with its batch sibling, then computes y = Q @ KtV and the
group norm for its rows.  The final swapaxes/reshape is pure data movement,
done on the host during unshard.

v2 changes vs the 241us baseline:
- The KtV partial + AllReduce is split into two D-half chunks (dh-major
  loop order), each triggered as soon as its V column-half is projected.
  Chunk 0's collective overlaps V-half-1 projection + KtV chunk 1 + the
  whole Q phase, so the PE no longer idles ~28us waiting for the exchange.
- KtV chunk fetches are issued after all weight/table DMAs (the SP engine
  runs its queue in order; an AR-dependent fetch issued earlier would
  head-block later weight loads).
- Weights and x are bf16 instead of f32r: same PE throughput, half the
  DMA traffic (~16MB -> 8MB of weight/x reads).
- PE warm-up trimmed 40 -> 8 junk matmuls (the HAM activity monitor
  budgets sustained PE activity; fake activity costs real throttle time).

Tricks kept from baseline:
- RoPE channel permutation: W_in's Q/K columns are pre-permuted (even
  channels then odd channels) on the host so the interleaved-pair rotation
  becomes contiguous-block arithmetic; the permutation cancels inside Q K^T.
- biases land via a pre-broadcast tensor add on the PSUM-evacuation pass
  (K/V) or an ACT Identity-activation per-partition bias (Q, transposed).
- K/V/Q activations, the KtV exchange, and the y matmul are bf16.
- group norm: stats from PSUM, then ygn = y*rstd (DVE) - mean*rstd (GpSimd).
"""

from contextlib import ExitStack

import ml_dtypes
import numpy as np
import concourse.bass as bass
import concourse.tile as tile
from concourse import bacc, mybir, bass_utils

B, T, D = 4, 2048, 1024
TL = T // 2          # rows per core (sequence half)
EPS = 1e-5
THETA = 10000.0
NGROUP = 16          # groups per 512-column tile
GSIZE = 32

F32 = mybir.dt.float32
F32R = mybir.dt.float32r
BF16 = mybir.dt.bfloat16

KV_DT = BF16          # dtype of K/V activations + KtV collective
RG = [[0, 1], [2, 3], [4, 5], [6, 7]]
RELU = mybir.ActivationFunctionType.Relu
IDENT = mybir.ActivationFunctionType.Identity

_CACHE = {}


def _build(gn_trivial):
    nc = bacc.Bacc("TRN2", target_bir_lowering=False, debug=False, num_devices=8)

    xT = nc.dram_tensor("xT", [D, TL], BF16, kind="ExternalInput")
    wq = nc.dram_tensor("wq", [D, D], BF16, kind="ExternalInput")
    wkv = nc.dram_tensor("wkv", [4 * D, 512], BF16, kind="ExternalInput")
    bq = nc.dram_tensor("bq", [128, 8], F32, kind="ExternalInput")
    bkvb = nc.dram_tensor("bkvb", [128, 2 * D], F32, kind="ExternalInput")
    cosq = nc.dram_tensor("cosq", [D // 2, TL], F32, kind="ExternalInput")
    sinq = nc.dram_tensor("sinq", [D // 2, TL], F32, kind="ExternalInput")
    cosk = nc.dram_tensor("cosk", [TL, D // 2], BF16, kind="ExternalInput")
    sink = nc.dram_tensor("sink", [TL, D // 2], BF16, kind="ExternalInput")
    if not gn_trivial:
        gnw = nc.dram_tensor("gnw", [128, D], F32, kind="ExternalInput")
        gnb = nc.dram_tensor("gnb", [128, D], F32, kind="ExternalInput")
    out = nc.dram_tensor("out", [2 * TL, 512], F32, kind="ExternalOutput")

    with tile.TileContext(nc) as tc, ExitStack() as st:
        psmall = st.enter_context(tc.tile_pool(name="small", bufs=1))
        pq = st.enter_context(tc.tile_pool(name="pq", bufs=1))
        pktv = st.enter_context(tc.tile_pool(name="pktv", bufs=1))
        pdram = st.enter_context(tc.tile_pool(name="pdram", bufs=1,
                                              space="DRAM"))

        bq_sb = psmall.tile([128, 8], F32, name="bq_sb")
        nc.sync.dma_start(bq_sb[:], bq[:])
        bkvb_sb = psmall.tile([128, 2 * D], F32, name="bkvb_sb")
        nc.sync.dma_start(bkvb_sb[:], bkvb[:])
        if not gn_trivial:
            gnw_sb = psmall.tile([128, D], F32, name="gnw_sb")
            nc.sync.dma_start(gnw_sb[:], gnw[:])
            gnb_sb = psmall.tile([128, D], F32, name="gnb_sb")
            nc.sync.dma_start(gnb_sb[:], gnb[:])
        eps_sb = psmall.tile([128, 1], F32, name="eps_sb")
        nc.vector.memset(eps_sb[:], EPS)

        qr = [pq.tile([128, TL], KV_DT, name=f"qr{j}", tag=f"qr{j}")
              for j in range(8)]
        ktv = [pktv.tile([128, D], KV_DT, name=f"ktv{c}", tag=f"ktv{c}")
               for c in range(8)]
        cc_out = [pdram.tile([D, 512], KV_DT, name=f"cco{dh}", tag=f"cco{dh}")
                  for dh in range(2)]

        with tc.tile_pool(name="pxt", bufs=1) as pxt, \
             tc.tile_pool(name="ppsA", bufs=8, space="PSUM") as ppsA:
            xt = [pxt.tile([128, TL], BF16, name=f"xt{d}", tag=f"xt{d}")
                  for d in range(8)]

            # ========= phase A1: K'/V proj, K rope, chunked KtV + collectives
            with tc.tile_pool(name="pkv", bufs=1) as pkv, \
                 tc.tile_pool(name="pwkv", bufs=12) as pwkv, \
                 tc.tile_pool(name="pkm", bufs=1) as pkm, \
                 tc.tile_pool(name="ptabk", bufs=3) as ptabk, \
                 tc.tile_pool(name="ptmpk", bufs=2) as ptmpk, \
                 tc.tile_pool(name="ppart", bufs=6) as ppart:

                kr = [pkv.tile([128, D], KV_DT, name=f"kr{t}", tag=f"kr{t}")
                      for t in range(8)]
                v_sb = [pkv.tile([128, D], KV_DT, name=f"v{t}", tag=f"v{t}")
                        for t in range(8)]
                km = [pkm.tile([128, D], KV_DT, name=f"km{t}", tag=f"km{t}")
                      for t in range(8)]

                def kv_proj(cb, tbs, wv):
                    for tb in tbs:
                        ps = ppsA.tile([128, 512], F32, name=f"pskv{cb}_{tb}",
                                       tag="psA")
                        for d in range(8):
                            nc.tensor.matmul(ps[:],
                                             xt[d][:, tb * 128:(tb + 1) * 128],
                                             wv[d][:], start=(d == 0),
                                             stop=(d == 7))
                        bias_s = bkvb_sb[:, cb * 512:(cb + 1) * 512]
                        if cb < 2:   # K' half -> km (+bias; rope next)
                            nc.vector.tensor_add(
                                km[tb][:, cb * 512:(cb + 1) * 512], ps[:],
                                bias_s)
                        else:        # V half -> +bias then relu in place
                            vslc = v_sb[tb][:, (cb - 2) * 512:(cb - 1) * 512]
                            nc.vector.tensor_add(vslc, ps[:], bias_s)
                            nc.scalar.activation(vslc, vslc, RELU)

                def load_wv(cb):
                    wv = [pwkv.tile([128, 512], BF16, name=f"wv{cb}_{d}",
                                    tag="wv") for d in range(8)]
                    for d in range(8):
                        if cb == 0:
                            nc.sync.dma_start(xt[d][:],
                                              xT[d * 128:(d + 1) * 128, :])
                        nc.sync.dma_start(
                            wv[d][:], wkv[cb * D + d * 128:
                                          cb * D + (d + 1) * 128, :])
                    return wv

                def ktv_chunk(dh):
                    """KtV partial for D-column half dh + its pair AllReduce.

                    The result lands in cc_out[dh]; the SBUF fetch is issued
                    later (after all weight/table DMAs) to avoid head-blocking
                    the SP DMA queue on the collective.
                    """
                    cc_in = pdram.tile([D, 512], KV_DT, name=f"cci{dh}",
                                       tag=f"cci{dh}")
                    for d1c in range(8):
                        ps = ppsA.tile([128, 512], F32,
                                       name=f"psk2_{dh}_{d1c}", tag="psA")
                        for tb in range(8):
                            nc.tensor.matmul(
                                ps[:],
                                kr[tb][:, d1c * 128:(d1c + 1) * 128],
                                v_sb[tb][:, dh * 512:(dh + 1) * 512],
                                start=(tb == 0), stop=(tb == 7))
                        part = ppart.tile([128, 512], KV_DT,
                                          name=f"part{dh}_{d1c}", tag="part")
                        if d1c % 2 == 0:
                            nc.vector.tensor_copy(part[:], ps[:])
                        else:
                            nc.scalar.copy(part[:], ps[:])
                        nc.sync.dma_start(cc_in[d1c * 128:(d1c + 1) * 128, :],
                                          part[:])
                    nc.gpsimd.collective_compute(
                        "AllReduce", mybir.AluOpType.add,
                        ins=[cc_in[:].opt()], outs=[cc_out[dh][:].opt()],
                        replica_groups=RG)

                # PE warm-up during the initial DMA fill keeps the HAM
                # activity monitor at full clock; kept short (fake activity
                # costs real throttle budget later)
                warm = ptmpk.tile([128, 512], F32R, name="warm", tag="warm",
                                  bufs=1)
                nc.vector.memset(warm[:].bitcast(F32), 0.0)
                wps = ppsA.tile([128, 512], F32, name="wps", tag="psA")
                for i in range(8):
                    nc.tensor.matmul(wps[:], warm[:, 0:128], warm[:],
                                     start=(i == 0), stop=(i == 7))

                # K' projection (cols 0:1024 of [K'|V]) then rope per row-block
                wv01 = [load_wv(0), load_wv(1)]
                kv_proj(0, range(8), wv01[0])
                kv_proj(1, range(8), wv01[1])
                for tb in range(8):
                    ck = ptabk.tile([128, 512], BF16, name=f"ck{tb}", tag="ck")
                    sk = ptabk.tile([128, 512], BF16, name=f"sk{tb}", tag="sk")
                    nc.sync.dma_start(ck[:], cosk[tb * 128:(tb + 1) * 128, :])
                    nc.sync.dma_start(sk[:], sink[tb * 128:(tb + 1) * 128, :])
                    x1 = km[tb][:, 0:512]
                    x2 = km[tb][:, 512:1024]
                    t1 = ptmpk.tile([128, 512], KV_DT, name=f"t1k{tb}",
                                    tag="t1")
                    t2 = ptmpk.tile([128, 512], KV_DT, name=f"t2k{tb}",
                                    tag="t2")
                    nc.vector.tensor_mul(t1[:], x1, sk[:])       # x1*sin
                    nc.vector.tensor_mul(x1, x1, ck[:])          # x1 = x1*cos
                    nc.vector.tensor_mul(t2[:], x2, sk[:])       # x2*sin
                    nc.vector.tensor_sub(x1, x1, t2[:])          # r1
                    nc.vector.tensor_mul(x2, x2, ck[:])          # x2 = x2*cos
                    nc.vector.tensor_add(x2, x2, t1[:])          # r2
                    nc.scalar.activation(kr[tb][:, 0:512], x1, RELU)
                    nc.scalar.activation(kr[tb][:, 512:1024], x2, RELU)

                # V half 0 -> KtV chunk 0 + its collective, then V half 1 ->
                # chunk 1; each collective overlaps the following PE work
                kv_proj(2, range(8), load_wv(2))
                ktv_chunk(0)
                kv_proj(3, range(8), load_wv(3))
                ktv_chunk(1)

            # ========= phase A2: Q' proj (transposed layout) + rope =========
            with tc.tile_pool(name="pwq", bufs=1) as pwq, \
                 tc.tile_pool(name="pqm", bufs=1) as pqm, \
                 tc.tile_pool(name="ptabq", bufs=4) as ptabq, \
                 tc.tile_pool(name="ptmpq", bufs=2) as ptmpq:

                wq_sb = [pwq.tile([128, D], BF16, name=f"wq{d}", tag=f"wq{d}")
                         for d in range(8)]
                for d in range(8):
                    nc.sync.dma_start(wq_sb[d][:],
                                      wq[d * 128:(d + 1) * 128, :])
                # all rope tables up-front: DMAs issued after this point may
                # legally wait on the collectives
                tabq = []
                for j in range(4):
                    cq = ptabq.tile([128, TL], F32, name=f"cq{j}", tag="cq")
                    sq = ptabq.tile([128, TL], F32, name=f"sq{j}", tag="sq")
                    nc.sync.dma_start(cq[:], cosq[j * 128:(j + 1) * 128, :])
                    nc.sync.dma_start(sq[:], sinq[j * 128:(j + 1) * 128, :])
                    tabq.append((cq, sq))

                # fetch the reduced KtV chunks straight into the y operand;
                # chunk 0 (the th=0 half) first so the y phase starts sooner
                for dh in range(2):
                    for d1c in range(8):
                        nc.sync.dma_start(
                            ktv[d1c][:, dh * 512:(dh + 1) * 512],
                            cc_out[dh][d1c * 128:(d1c + 1) * 128, :])

                qm = [pqm.tile([128, TL], F32, name=f"qm{j}", tag=f"qm{j}")
                      for j in range(8)]

                def q_proj(cp):
                    for th in range(2):
                        ps = ppsA.tile([128, 512], F32, name=f"psq{cp}_{th}",
                                       tag="psA")
                        for d in range(8):
                            nc.tensor.matmul(
                                ps[:], wq_sb[d][:, cp * 128:(cp + 1) * 128],
                                xt[d][:, th * 512:(th + 1) * 512],
                                start=(d == 0), stop=(d == 7))
                        nc.scalar.activation(
                            qm[cp][:, th * 512:(th + 1) * 512], ps[:],
                            IDENT, bias=bq_sb[:, cp:cp + 1])

                # pair (j, j+4) projected together so rope(j) starts while the
                # next pair is still on the PE
                for j in range(4):
                    q_proj(j)
                    q_proj(j + 4)
                    cq, sq = tabq[j]
                    x1 = qm[j][:]
                    x2 = qm[j + 4][:]
                    t1 = ptmpq.tile([128, TL], F32, name=f"t1q{j}", tag="t1",
                                    bufs=1)
                    t2 = ptmpq.tile([128, TL], F32, name=f"t2q{j}", tag="t2",
                                    bufs=1)
                    nc.gpsimd.tensor_mul(t1[:], x1, sq[:])       # x1*sin
                    nc.vector.tensor_mul(x1, x1, cq[:])          # x1*cos
                    nc.vector.tensor_mul(t2[:], x2, sq[:])       # x2*sin
                    nc.vector.tensor_sub(x1, x1, t2[:])          # r1
                    nc.vector.tensor_mul(x2, x2, cq[:])          # x2*cos
                    nc.vector.tensor_add(x2, x2, t1[:])          # r2
                    nc.scalar.activation(qr[j][:], x1, RELU)
                    nc.scalar.activation(qr[j + 4][:], x2, RELU)

        # ================= phase C: y = Q' @ KtV + fused group norm ========
        with tc.tile_pool(name="pgn", bufs=4) as pgn, \
             tc.tile_pool(name="pstat", bufs=4) as pstat, \
             tc.tile_pool(name="ppsY", bufs=1, space="PSUM") as ppsY:
            inv32 = 1.0 / GSIZE
            for th in range(2):
                for tb in range(8):
                    ps = ppsY.tile([128, 512], F32, name=f"psy{th}_{tb}",
                                   tag=f"psy{tb % 4}")
                    for c in range(8):
                        nc.tensor.matmul(
                            ps[:], qr[c][:, tb * 128:(tb + 1) * 128],
                            ktv[c][:, th * 512:(th + 1) * 512],
                            start=(c == 0), stop=(c == 7))
                    ps3 = ps[:].rearrange("p (g c) -> p g c", g=NGROUP)
                    sums = pstat.tile([128, 16], F32, name=f"su{th}_{tb}",
                                      tag="su")
                    nc.vector.reduce_sum(sums[:], ps3,
                                         axis=mybir.AxisListType.X)
                    sqt = pgn.tile([128, 512], F32, name=f"sqt{th}_{tb}",
                                   tag="sqt")
                    nc.scalar.square(sqt[:], ps[:])
                    sums2 = pstat.tile([128, 16], F32, name=f"s2{th}_{tb}",
                                       tag="s2")
                    nc.vector.reduce_sum(
                        sums2[:],
                        sqt[:].rearrange("p (g c) -> p g c", g=NGROUP),
                        axis=mybir.AxisListType.X)
                    mean = pstat.tile([128, 16], F32, name=f"mn{th}_{tb}",
                                      tag="mn")
                    nc.vector.tensor_scalar_mul(mean[:], sums[:], inv32)
                    ex2 = pstat.tile([128, 16], F32, name=f"e2{th}_{tb}",
                                     tag="e2")
                    nc.gpsimd.tensor_scalar_mul(ex2[:], sums2[:], inv32)
                    msq = pstat.tile([128, 16], F32, name=f"mq{th}_{tb}",
                                     tag="mq")
                    nc.vector.tensor_mul(msq[:], mean[:], mean[:])
                    var = pstat.tile([128, 16], F32, name=f"va{th}_{tb}",
                                     tag="va")
                    nc.vector.tensor_sub(var[:], ex2[:], msq[:])
                    sd = pstat.tile([128, 16], F32, name=f"sd{th}_{tb}",
                                    tag="sd")
                    nc.scalar.activation(sd[:], var[:],
                                         mybir.ActivationFunctionType.Sqrt,
                                         bias=eps_sb[:])
                    rstd = pstat.tile([128, 16], F32, name=f"rs{th}_{tb}",
                                      tag="rs")
                    nc.vector.reciprocal(rstd[:], sd[:])
                    mr = pstat.tile([128, 16], F32, name=f"mr{th}_{tb}",
                                    tag="mr")
                    nc.vector.tensor_mul(mr[:], mean[:], rstd[:])

                    ygn = pgn.tile([128, 512], F32, name=f"yg{th}_{tb}",
                                   tag="ygn")
                    y3 = ygn[:].rearrange("p (g c) -> p g c", g=NGROUP)
                    nc.vector.tensor_mul(
                        y3, ps3, rstd[:].broadcast_to([128, NGROUP, GSIZE]))
                    nc.gpsimd.tensor_sub(
                        y3, y3, mr[:].broadcast_to([128, NGROUP, GSIZE]))
                    cs = slice(th * 512, (th + 1) * 512)
                    if not gn_trivial:
                        nc.gpsimd.tensor_mul(ygn[:], ygn[:], gnw_sb[:, cs])
                        nc.gpsimd.tensor_add(ygn[:], ygn[:], gnb_sb[:, cs])
                    ro = th * TL + tb * 128
                    nc.sync.dma_start(out[ro:ro + 128, :], ygn[:])

    nc.compile()
    return nc


def _get_nc(gn_trivial):
    key = ("nc", gn_trivial)
    if key not in _CACHE:
        _CACHE[key] = _build(gn_trivial)
    return _CACHE[key]


def _make_in_maps(x, W_in, b_in, gn_weight, gn_bias, gn_trivial):
    perm = np.concatenate([np.arange(0, D, 2), np.arange(1, D, 2)])
    wq_h = np.ascontiguousarray(W_in[:, 0:D][:, perm]).astype(
        ml_dtypes.bfloat16)
    wk = W_in[:, D:2 * D][:, perm]
    wv = W_in[:, 2 * D:3 * D]
    wkv_cat = np.concatenate([wk, wv], axis=1)          # [D, 2D]
    wkv_h = np.ascontiguousarray(
        np.concatenate([wkv_cat[:, cb * 512:(cb + 1) * 512]
                        for cb in range(4)], axis=0)).astype(
        ml_dtypes.bfloat16)                              # [4D, 512]
    bq_h = np.ascontiguousarray(b_in[0:D][perm].reshape(8, 128).T)
    bkv_row = np.concatenate([b_in[D:2 * D][perm], b_in[2 * D:3 * D]])
    bkvb_h = np.ascontiguousarray(
        np.broadcast_to(bkv_row[None, :], (128, 2 * D))).astype(np.float32)
    gnw_h = np.ascontiguousarray(
        np.broadcast_to(np.float32(gn_weight)[None, :], (128, D)))
    gnb_h = np.ascontiguousarray(
        np.broadcast_to(np.float32(gn_bias)[None, :], (128, D)))

    inv_freq = (1.0 / (THETA ** (np.arange(0, D, 2, dtype=np.float32) / D))
                ).astype(np.float32)

    in_maps = []
    for core in range(8):
        b, h = divmod(core, 2)
        ts = np.arange(h * TL, (h + 1) * TL, dtype=np.float32)
        freqs = ts[:, None] * inv_freq[None, :]      # [TL, 512]
        cos_n = np.cos(freqs).astype(np.float32)
        sin_n = np.sin(freqs).astype(np.float32)
        xT_h = np.ascontiguousarray(
            x[b, h * TL:(h + 1) * TL, :].T).astype(ml_dtypes.bfloat16)
        m = {
            "xT": xT_h, "wq": wq_h, "wkv": wkv_h, "bq": bq_h, "bkvb": bkvb_h,
            "cosq": np.ascontiguousarray(cos_n.T),
            "sinq": np.ascontiguousarray(sin_n.T),
            "cosk": cos_n.astype(ml_dtypes.bfloat16),
            "sink": sin_n.astype(ml_dtypes.bfloat16),
        }
        if not gn_trivial:
            m["gnw"] = gnw_h
            m["gnb"] = gnb_h
        in_maps.append(m)
    return in_maps


def kernel(x, W_in, b_in, gn_weight, gn_bias, _trace=False):
    x = np.asarray(x, dtype=np.float32)
    W_in = np.asarray(W_in, dtype=np.float32)
    b_in = np.asarray(b_in, dtype=np.float32)
    gn_weight = np.asarray(gn_weight, dtype=np.float32)
    gn_bias = np.asarray(gn_bias, dtype=np.float32)

    gn_trivial = bool(np.all(gn_weight == 1.0) and np.all(gn_bias == 0.0))
    nc = _get_nc(gn_trivial)
    in_maps = _make_in_maps(x, W_in, b_in, gn_weight, gn_bias, gn_trivial)
    res = bass_utils.run_bass_kernel_spmd(nc, in_maps, core_ids=list(range(8)),
                                          trace=_trace)
    _CACHE["last_result"] = res

    outs = [np.concatenate([res.results[i]["out"][:TL],
                            res.results[i]["out"][TL:]], axis=1)
            for i in range(8)]                            # [TL, D] each
    full = np.empty((B, T, D), dtype=np.float32)
    for b in range(B):
        y_gn = np.concatenate([outs[2 * b], outs[2 * b + 1]], axis=0)  # [T,D]
        full[b] = y_gn.T.reshape(T, D)
    return full


# revision 4
# speedup vs baseline: 1.2542x; 1.0868x over previous
"""Trainium2 Bass kernel for ActivatedAttention (B=4, T=2048, D=1024, f32).

  qkv = x @ W_in + b_in;  Q,K,V = split(qkv)
  Q = relu(rope(Q)); K = relu(rope(K)); V = relu(V)
  y = (Q @ K^T) @ V            # no softmax -> reassociate: y = Q @ (K^T @ V)
  out = swapaxes(group_norm(y), -2, -1).reshape(B, T, D)

Sharding: 8 cores = (batch b in 0..3) x (sequence half h in 0..1).
Each core projects its own 1024 rows, computes its partial K^T V, pair-
AllReduces it with its batch sibling, then computes y = Q @ KtV and the
group norm for its rows.  The final swapaxes/reshape is pure data movement,
done on the host during unshard.

v3 changes vs the 210us v2:
- wq + Q rope-table DMAs issue during phase A1 (DMA slack exists there);
  v2 issued them at A2 entry, leaving the PE idle ~9.5us waiting for wq.
- K rope-table DMAs issue before the K projection for the same reason.
- Phase C uses all 8 PSUM banks (v2 used 4, stalling the PE on
  write-after-read vs the group-norm drain).
- Group-norm stat work rebalanced across scalar/vector/gpsimd so no one
  engine backlogs; y/ygn path and the Q rope are bf16 (2x DVE rate), and
  the output tensor is bf16 (half the drain DMA), cast to f32 on host.

v2 changes vs the 241us baseline:
- KtV partial + AllReduce split into two D-half chunks (dh-major), each
  triggered as soon as its V column-half is projected; the exchange hides
  under V-half-1 projection + KtV chunk 1 + the whole Q phase.
- KtV chunk fetches issue after all weight/table DMAs (the SP engine runs
  its queue in order; an AR-dependent fetch would head-block weight loads).
- Weights and x are bf16 instead of f32r: same PE throughput, half DMA.
- PE warm-up trimmed 40 -> 8 junk matmuls (HAM activity budget).

Tricks kept from baseline:
- RoPE channel permutation: W_in's Q/K columns are pre-permuted (even
  channels then odd channels) on the host so the interleaved-pair rotation
  becomes contiguous-block arithmetic; the permutation cancels inside Q K^T.
- biases land via a pre-broadcast tensor add on the PSUM-evacuation pass
  (K/V) or an ACT Identity-activation per-partition bias (Q, transposed).
- group norm: stats from PSUM, then ygn = y*rstd - mean*rstd.
"""

from contextlib import ExitStack

import ml_dtypes
import numpy as np
import concourse.bass as bass
import concourse.tile as tile
from concourse import bacc, mybir, bass_utils

B, T, D = 4, 2048, 1024
TL = T // 2          # rows per core (sequence half)
EPS = 1e-5
THETA = 10000.0
NGROUP = 16          # groups per 512-column tile
GSIZE = 32

F32 = mybir.dt.float32
F32R = mybir.dt.float32r
BF16 = mybir.dt.bfloat16

KV_DT = BF16          # dtype of K/V activations + KtV collective
RG = [[0, 1], [2, 3], [4, 5], [6, 7]]
RELU = mybir.ActivationFunctionType.Relu
IDENT = mybir.ActivationFunctionType.Identity

_CACHE = {}


def _build(gn_trivial):
    nc = bacc.Bacc("TRN2", target_bir_lowering=False, debug=False, num_devices=8)

    xT = nc.dram_tensor("xT", [D, TL], BF16, kind="ExternalInput")
    wq = nc.dram_tensor("wq", [D, D], BF16, kind="ExternalInput")
    wkv = nc.dram_tensor("wkv", [4 * D, 512], BF16, kind="ExternalInput")
    bq = nc.dram_tensor("bq", [128, 8], F32, kind="ExternalInput")
    bkvb = nc.dram_tensor("bkvb", [128, 2 * D], F32, kind="ExternalInput")
    cosq = nc.dram_tensor("cosq", [D // 2, TL], BF16, kind="ExternalInput")
    sinq = nc.dram_tensor("sinq", [D // 2, TL], BF16, kind="ExternalInput")
    cosk = nc.dram_tensor("cosk", [TL, D // 2], BF16, kind="ExternalInput")
    sink = nc.dram_tensor("sink", [TL, D // 2], BF16, kind="ExternalInput")
    if not gn_trivial:
        gnw = nc.dram_tensor("gnw", [128, D], F32, kind="ExternalInput")
        gnb = nc.dram_tensor("gnb", [128, D], F32, kind="ExternalInput")
    out = nc.dram_tensor("out", [2 * TL, 512], BF16, kind="ExternalOutput")

    with tile.TileContext(nc) as tc, ExitStack() as st:
        psmall = st.enter_context(tc.tile_pool(name="small", bufs=1))
        pq = st.enter_context(tc.tile_pool(name="pq", bufs=1))
        pktv = st.enter_context(tc.tile_pool(name="pktv", bufs=1))
        pwq = st.enter_context(tc.tile_pool(name="pwq", bufs=1))
        ptabq = st.enter_context(tc.tile_pool(name="ptabq", bufs=1))
        pdram = st.enter_context(tc.tile_pool(name="pdram", bufs=1,
                                              space="DRAM"))

        bq_sb = psmall.tile([128, 8], F32, name="bq_sb")
        nc.sync.dma_start(bq_sb[:], bq[:])
        bkvb_sb = psmall.tile([128, 2 * D], F32, name="bkvb_sb")
        nc.sync.dma_start(bkvb_sb[:], bkvb[:])
        if not gn_trivial:
            gnw_sb = psmall.tile([128, D], F32, name="gnw_sb")
            nc.sync.dma_start(gnw_sb[:], gnw[:])
            gnb_sb = psmall.tile([128, D], F32, name="gnb_sb")
            nc.sync.dma_start(gnb_sb[:], gnb[:])
        eps_sb = psmall.tile([128, 1], F32, name="eps_sb")
        nc.vector.memset(eps_sb[:], EPS)

        qr = [pq.tile([128, TL], KV_DT, name=f"qr{j}", tag=f"qr{j}")
              for j in range(8)]
        ktv = [pktv.tile([128, D], KV_DT, name=f"ktv{c}", tag=f"ktv{c}")
               for c in range(8)]
        wq_sb = [pwq.tile([128, D], BF16, name=f"wq{d}", tag=f"wq{d}")
                 for d in range(8)]
        tabq = [(ptabq.tile([128, TL], BF16, name=f"cq{j}", tag=f"cq{j}"),
                 ptabq.tile([128, TL], BF16, name=f"sq{j}", tag=f"sq{j}"))
                for j in range(4)]
        cc_out = [pdram.tile([D, 512], KV_DT, name=f"cco{dh}", tag=f"cco{dh}")
                  for dh in range(2)]

        with tc.tile_pool(name="pxt", bufs=1) as pxt, \
             tc.tile_pool(name="ppsA", bufs=8, space="PSUM") as ppsA:
            xt = [pxt.tile([128, TL], BF16, name=f"xt{d}", tag=f"xt{d}")
                  for d in range(8)]

            # ========= phase A1: K'/V proj, K rope, chunked KtV + collectives
            with tc.tile_pool(name="pkv", bufs=1) as pkv, \
                 tc.tile_pool(name="pwkv", bufs=12) as pwkv, \
                 tc.tile_pool(name="pkm", bufs=1) as pkm, \
                 tc.tile_pool(name="ptabk", bufs=8) as ptabk, \
                 tc.tile_pool(name="ptmpk", bufs=2) as ptmpk, \
                 tc.tile_pool(name="ppart", bufs=6) as ppart:

                kr = [pkv.tile([128, D], KV_DT, name=f"kr{t}", tag=f"kr{t}")
                      for t in range(8)]
                v_sb = [pkv.tile([128, D], KV_DT, name=f"v{t}", tag=f"v{t}")
                        for t in range(8)]
                km = [pkm.tile([128, D], KV_DT, name=f"km{t}", tag=f"km{t}")
                      for t in range(8)]

                def kv_proj(cb, tbs, wv):
                    for tb in tbs:
                        ps = ppsA.tile([128, 512], F32, name=f"pskv{cb}_{tb}",
                                       tag="psA")
                        for d in range(8):
                            nc.tensor.matmul(ps[:],
                                             xt[d][:, tb * 128:(tb + 1) * 128],
                                             wv[d][:], start=(d == 0),
                                             stop=(d == 7))
                        bias_s = bkvb_sb[:, cb * 512:(cb + 1) * 512]
                        if cb < 2:   # K' half -> km (+bias; rope next)
                            nc.vector.tensor_add(
                                km[tb][:, cb * 512:(cb + 1) * 512], ps[:],
                                bias_s)
                        else:        # V half -> +bias then relu in place
                            vslc = v_sb[tb][:, (cb - 2) * 512:(cb - 1) * 512]
                            nc.vector.tensor_add(vslc, ps[:], bias_s)
                            nc.scalar.activation(vslc, vslc, RELU)

                def load_wv(cb):
                    wv = [pwkv.tile([128, 512], BF16, name=f"wv{cb}_{d}",
                                    tag="wv") for d in range(8)]
                    for d in range(8):
                        if cb == 0:
                            nc.sync.dma_start(xt[d][:],
                                              xT[d * 128:(d + 1) * 128, :])
                        nc.sync.dma_start(
                            wv[d][:], wkv[cb * D + d * 128:
                                          cb * D + (d + 1) * 128, :])
                    return wv

                def ktv_chunk(dh):
                    """KtV partial for D-column half dh + its pair AllReduce.

                    The result lands in cc_out[dh]; the SBUF fetch is issued
                    later (after all weight/table DMAs) to avoid head-blocking
                    the SP DMA queue on the collective.
                    """
                    cc_in = pdram.tile([D, 512], KV_DT, name=f"cci{dh}",
                                       tag=f"cci{dh}")
                    for d1c in range(8):
                        ps = ppsA.tile([128, 512], F32,
                                       name=f"psk2_{dh}_{d1c}", tag="psA")
                        for tb in range(8):
                            nc.tensor.matmul(
                                ps[:],
                                kr[tb][:, d1c * 128:(d1c + 1) * 128],
                                v_sb[tb][:, dh * 512:(dh + 1) * 512],
                                start=(tb == 0), stop=(tb == 7))
                        part = ppart.tile([128, 512], KV_DT,
                                          name=f"part{dh}_{d1c}", tag="part")
                        if d1c % 2 == 0:
                            nc.vector.tensor_copy(part[:], ps[:])
                        else:
                            nc.scalar.copy(part[:], ps[:])
                        nc.sync.dma_start(cc_in[d1c * 128:(d1c + 1) * 128, :],
                                          part[:])
                    nc.gpsimd.collective_compute(
                        "AllReduce", mybir.AluOpType.add,
                        ins=[cc_in[:].opt()], outs=[cc_out[dh][:].opt()],
                        replica_groups=RG)

                # PE warm-up during the initial DMA fill keeps the HAM
                # activity monitor at full clock; kept short (fake activity
                # costs real throttle budget later)
                warm = ptmpk.tile([128, 512], F32R, name="warm", tag="warm",
                                  bufs=1)
                nc.vector.memset(warm[:].bitcast(F32), 0.0)
                wps = ppsA.tile([128, 512], F32, name="wps", tag="psA")
                for i in range(8):
                    nc.tensor.matmul(wps[:], warm[:, 0:128], warm[:],
                                     start=(i == 0), stop=(i == 7))

                # K' projection (cols 0:1024 of [K'|V]) then rope per row-block
                wv01 = [load_wv(0), load_wv(1)]
                ktab = []
                for tb in range(8):
                    ck = ptabk.tile([128, 512], BF16, name=f"ck{tb}", tag="ck")
                    sk = ptabk.tile([128, 512], BF16, name=f"sk{tb}", tag="sk")
                    nc.sync.dma_start(ck[:], cosk[tb * 128:(tb + 1) * 128, :])
                    nc.sync.dma_start(sk[:], sink[tb * 128:(tb + 1) * 128, :])
                    ktab.append((ck, sk))
                kv_proj(0, range(8), wv01[0])
                kv_proj(1, range(8), wv01[1])
                for tb in range(8):
                    ck, sk = ktab[tb]
                    x1 = km[tb][:, 0:512]
                    x2 = km[tb][:, 512:1024]
                    t1 = ptmpk.tile([128, 512], KV_DT, name=f"t1k{tb}",
                                    tag="t1")
                    t2 = ptmpk.tile([128, 512], KV_DT, name=f"t2k{tb}",
                                    tag="t2")
                    nc.vector.tensor_mul(t1[:], x1, sk[:])       # x1*sin
                    nc.vector.tensor_mul(x1, x1, ck[:])          # x1 = x1*cos
                    nc.vector.tensor_mul(t2[:], x2, sk[:])       # x2*sin
                    nc.vector.tensor_sub(x1, x1, t2[:])          # r1
                    nc.vector.tensor_mul(x2, x2, ck[:])          # x2 = x2*cos
                    nc.vector.tensor_add(x2, x2, t1[:])          # r2
                    nc.scalar.activation(kr[tb][:, 0:512], x1, RELU)
                    nc.scalar.activation(kr[tb][:, 512:1024], x2, RELU)

                # V half 0 -> KtV chunk 0 + its collective, then V half 1 ->
                # chunk 1; each collective overlaps the following PE work.
                # wq + Q-table DMAs issue here: there is DMA slack, and A2
                # must not wait on them.
                kv_proj(2, range(8), load_wv(2))
                ktv_chunk(0)
                wv3 = load_wv(3)
                for d in range(8):
                    nc.sync.dma_start(wq_sb[d][:],
                                      wq[d * 128:(d + 1) * 128, :])
                for j in range(4):
                    cq, sq = tabq[j]
                    nc.sync.dma_start(cq[:], cosq[j * 128:(j + 1) * 128, :])
                    nc.sync.dma_start(sq[:], sinq[j * 128:(j + 1) * 128, :])
                kv_proj(3, range(8), wv3)
                ktv_chunk(1)

            # ========= phase A2: Q' proj (transposed layout) + rope =========
            with tc.tile_pool(name="pqm", bufs=1) as pqm, \
                 tc.tile_pool(name="ptmpq", bufs=2) as ptmpq:

                # fetch the reduced KtV chunks straight into the y operand;
                # chunk 0 (the th=0 half) first so the y phase starts sooner.
                # All load-bearing DMAs are already issued, so blocking the
                # SP queue on the collectives is harmless.
                for dh in range(2):
                    for d1c in range(8):
                        nc.sync.dma_start(
                            ktv[d1c][:, dh * 512:(dh + 1) * 512],
                            cc_out[dh][d1c * 128:(d1c + 1) * 128, :])

                qm = [pqm.tile([128, TL], BF16, name=f"qm{j}", tag=f"qm{j}")
                      for j in range(8)]

                def q_proj(cp):
                    for th in range(2):
                        ps = ppsA.tile([128, 512], F32, name=f"psq{cp}_{th}",
                                       tag="psA")
                        for d in range(8):
                            nc.tensor.matmul(
                                ps[:], wq_sb[d][:, cp * 128:(cp + 1) * 128],
                                xt[d][:, th * 512:(th + 1) * 512],
                                start=(d == 0), stop=(d == 7))
                        nc.scalar.activation(
                            qm[cp][:, th * 512:(th + 1) * 512], ps[:],
                            IDENT, bias=bq_sb[:, cp:cp + 1])

                # pair (j, j+4) projected together so rope(j) starts while the
                # next pair is still on the PE
                for j in range(4):
                    q_proj(j)
                    q_proj(j + 4)
                    cq, sq = tabq[j]
                    x1 = qm[j][:]
                    x2 = qm[j + 4][:]
                    t1 = ptmpq.tile([128, TL], BF16, name=f"t1q{j}", tag="t1",
                                    bufs=1)
                    t2 = ptmpq.tile([128, TL], BF16, name=f"t2q{j}", tag="t2",
                                    bufs=1)
                    nc.gpsimd.tensor_mul(t1[:], x1, sq[:])       # x1*sin
                    nc.vector.tensor_mul(x1, x1, cq[:])          # x1*cos
                    nc.vector.tensor_mul(t2[:], x2, sq[:])       # x2*sin
                    nc.vector.tensor_sub(x1, x1, t2[:])          # r1
                    nc.vector.tensor_mul(x2, x2, cq[:])          # x2*cos
                    nc.vector.tensor_add(x2, x2, t1[:])          # r2
                    nc.scalar.activation(qr[j][:], x1, RELU)
                    nc.scalar.activation(qr[j + 4][:], x2, RELU)

        # ================= phase C: y = Q' @ KtV + fused group norm ========
        # stat chain is spread over scalar/vector/gpsimd so no single engine
        # backlogs behind the PE
        with tc.tile_pool(name="pgn", bufs=4) as pgn, \
             tc.tile_pool(name="pstat", bufs=4) as pstat, \
             tc.tile_pool(name="ppsY", bufs=1, space="PSUM") as ppsY:
            inv32 = 1.0 / GSIZE
            for th in range(2):
                for tb in range(8):
                    ps = ppsY.tile([128, 512], F32, name=f"psy{th}_{tb}",
                                   tag=f"psy{tb}")
                    for c in range(8):
                        nc.tensor.matmul(
                            ps[:], qr[c][:, tb * 128:(tb + 1) * 128],
                            ktv[c][:, th * 512:(th + 1) * 512],
                            start=(c == 0), stop=(c == 7))
                    ps3 = ps[:].rearrange("p (g c) -> p g c", g=NGROUP)
                    sums = pstat.tile([128, 16], F32, name=f"su{th}_{tb}",
                                      tag="su")
                    nc.vector.reduce_sum(sums[:], ps3,
                                         axis=mybir.AxisListType.X)
                    sqt = pgn.tile([128, 512], F32, name=f"sqt{th}_{tb}",
                                   tag="sqt")
                    nc.scalar.square(sqt[:], ps[:])
                    sums2 = pstat.tile([128, 16], F32, name=f"s2{th}_{tb}",
                                       tag="s2")
                    nc.vector.reduce_sum(
                        sums2[:],
                        sqt[:].rearrange("p (g c) -> p g c", g=NGROUP),
                        axis=mybir.AxisListType.X)
                    mean = pstat.tile([128, 16], F32, name=f"mn{th}_{tb}",
                                      tag="mn")
                    nc.scalar.activation(mean[:], sums[:], IDENT, scale=inv32)
                    ex2 = pstat.tile([128, 16], F32, name=f"e2{th}_{tb}",
                                     tag="e2")
                    nc.gpsimd.tensor_scalar_mul(ex2[:], sums2[:], inv32)
                    msq = pstat.tile([128, 16], F32, name=f"mq{th}_{tb}",
                                     tag="mq")
                    nc.gpsimd.tensor_mul(msq[:], mean[:], mean[:])
                    var = pstat.tile([128, 16], F32, name=f"va{th}_{tb}",
                                     tag="va")
                    nc.gpsimd.tensor_sub(var[:], ex2[:], msq[:])
                    sd = pstat.tile([128, 16], F32, name=f"sd{th}_{tb}",
                                    tag="sd")
                    nc.scalar.activation(sd[:], var[:],
                                         mybir.ActivationFunctionType.Sqrt,
                                         bias=eps_sb[:])
                    rstd = pstat.tile([128, 16], F32, name=f"rs{th}_{tb}",
                                      tag="rs")
                    nc.vector.reciprocal(rstd[:], sd[:])
                    mr = pstat.tile([128, 16], F32, name=f"mr{th}_{tb}",
                                    tag="mr")
                    nc.vector.tensor_mul(mr[:], mean[:], rstd[:])

                    # y*rstd stays f32: its magnitude is ~mean/std (large),
                    # so a bf16 rounding BEFORE the mean subtraction would be
                    # amplified relative to the normalized output. Only the
                    # final subtract emits bf16.
                    ygn = pgn.tile([128, 512], F32, name=f"yg{th}_{tb}",
                                   tag="ygn")
                    y3 = ygn[:].rearrange("p (g c) -> p g c", g=NGROUP)
                    nc.vector.tensor_mul(
                        y3, ps3, rstd[:].broadcast_to([128, NGROUP, GSIZE]))
                    yout = pgn.tile([128, 512], KV_DT, name=f"yo{th}_{tb}",
                                    tag="yout")
                    yo3 = yout[:].rearrange("p (g c) -> p g c", g=NGROUP)
                    cs = slice(th * 512, (th + 1) * 512)
                    if gn_trivial:
                        nc.gpsimd.tensor_sub(
                            yo3, y3, mr[:].broadcast_to([128, NGROUP, GSIZE]))
                    else:
                        nc.gpsimd.tensor_sub(
                            y3, y3, mr[:].broadcast_to([128, NGROUP, GSIZE]))
                        nc.gpsimd.tensor_mul(ygn[:], ygn[:], gnw_sb[:, cs])
                        nc.gpsimd.tensor_add(yout[:], ygn[:], gnb_sb[:, cs])
                    ro = th * TL + tb * 128
                    nc.sync.dma_start(out[ro:ro + 128, :], yout[:])

    nc.compile()
    return nc


def _get_nc(gn_trivial):
    key = ("nc", gn_trivial)
    if key not in _CACHE:
        _CACHE[key] = _build(gn_trivial)
    return _CACHE[key]


def _make_in_maps(x, W_in, b_in, gn_weight, gn_bias, gn_trivial):
    perm = np.concatenate([np.arange(0, D, 2), np.arange(1, D, 2)])
    wq_h = np.ascontiguousarray(W_in[:, 0:D][:, perm]).astype(
        ml_dtypes.bfloat16)
    wk = W_in[:, D:2 * D][:, perm]
    wv = W_in[:, 2 * D:3 * D]
    wkv_cat = np.concatenate([wk, wv], axis=1)          # [D, 2D]
    wkv_h = np.ascontiguousarray(
        np.concatenate([wkv_cat[:, cb * 512:(cb + 1) * 512]
                        for cb in range(4)], axis=0)).astype(
        ml_dtypes.bfloat16)                              # [4D, 512]
    bq_h = np.ascontiguousarray(b_in[0:D][perm].reshape(8, 128).T)
    bkv_row = np.concatenate([b_in[D:2 * D][perm], b_in[2 * D:3 * D]])
    bkvb_h = np.ascontiguousarray(
        np.broadcast_to(bkv_row[None, :], (128, 2 * D))).astype(np.float32)
    gnw_h = np.ascontiguousarray(
        np.broadcast_to(np.float32(gn_weight)[None, :], (128, D)))
    gnb_h = np.ascontiguousarray(
        np.broadcast_to(np.float32(gn_bias)[None, :], (128, D)))

    inv_freq = (1.0 / (THETA ** (np.arange(0, D, 2, dtype=np.float32) / D))
                ).astype(np.float32)

    in_maps = []
    for core in range(8):
        b, h = divmod(core, 2)
        ts = np.arange(h * TL, (h + 1) * TL, dtype=np.float32)
        freqs = ts[:, None] * inv_freq[None, :]      # [TL, 512]
        cos_n = np.cos(freqs).astype(np.float32)
        sin_n = np.sin(freqs).astype(np.float32)
        xT_h = np.ascontiguousarray(
            x[b, h * TL:(h + 1) * TL, :].T).astype(ml_dtypes.bfloat16)
        m = {
            "xT": xT_h, "wq": wq_h, "wkv": wkv_h, "bq": bq_h, "bkvb": bkvb_h,
            "cosq": np.ascontiguousarray(cos_n.T).astype(ml_dtypes.bfloat16),
            "sinq": np.ascontiguousarray(sin_n.T).astype(ml_dtypes.bfloat16),
            "cosk": cos_n.astype(ml_dtypes.bfloat16),
            "sink": sin_n.astype(ml_dtypes.bfloat16),
        }
        if not gn_trivial:
            m["gnw"] = gnw_h
            m["gnb"] = gnb_h
        in_maps.append(m)
    return in_maps


def kernel(x, W_in, b_in, gn_weight, gn_bias, _trace=False):
    x = np.asarray(x, dtype=np.float32)
    W_in = np.asarray(W_in, dtype=np.float32)
    b_in = np.asarray(b_in, dtype=np.float32)
    gn_weight = np.asarray(gn_weight, dtype=np.float32)
    gn_bias = np.asarray(gn_bias, dtype=np.float32)

    gn_trivial = bool(np.all(gn_weight == 1.0) and np.all(gn_bias == 0.0))
    nc = _get_nc(gn_trivial)
    in_maps = _make_in_maps(x, W_in, b_in, gn_weight, gn_bias, gn_trivial)
    res = bass_utils.run_bass_kernel_spmd(nc, in_maps, core_ids=list(range(8)),
                                          trace=_trace)
    _CACHE["last_result"] = res

    outs = [np.concatenate([np.float32(res.results[i]["out"][:TL]),
                            np.float32(res.results[i]["out"][TL:])], axis=1)
            for i in range(8)]                            # [TL, D] each
    full = np.empty((B, T, D), dtype=np.float32)
    for b in range(B):
        y_gn = np.concatenate([outs[2 * b], outs[2 * b + 1]], axis=0)  # [T,D]
        full[b] = y_gn.T.reshape(T, D)
    return full


# revision 8
# speedup vs baseline: 1.3170x; 1.0501x over previous
"""Trainium2 Bass kernel for ActivatedAttention (B=4, T=2048, D=1024, f32).

  qkv = x @ W_in + b_in;  Q,K,V = split(qkv)
  Q = relu(rope(Q)); K = relu(rope(K)); V = relu(V)
  y = (Q @ K^T) @ V            # no softmax -> reassociate: y = Q @ (K^T @ V)
  out = swapaxes(group_norm(y), -2, -1).reshape(B, T, D)

Sharding: 8 cores = (batch b in 0..3) x (sequence half h in 0..1).
Each core projects its own 1024 rows, computes its partial K^T V, pair-
AllReduces it with its batch sibling, then computes y = Q @ KtV and the
group norm for its rows.  The final swapaxes/reshape is pure data movement,
done on the host during unshard.

v3 changes vs the 210us v2:
- wq + Q rope-table DMAs issue during phase A1 (DMA slack exists there);
  v2 issued them at A2 entry, leaving the PE idle ~9.5us waiting for wq.
- K rope-table DMAs issue before the K projection for the same reason.
- Phase C uses all 8 PSUM banks (v2 used 4, stalling the PE on
  write-after-read vs the group-norm drain).
- Group-norm stat work rebalanced across scalar/vector/gpsimd so no one
  engine backlogs; y/ygn path and the Q rope are bf16 (2x DVE rate), and
  the output tensor is bf16 (half the drain DMA), cast to f32 on host.

v2 changes vs the 241us baseline:
- KtV partial + AllReduce split into two D-half chunks (dh-major), each
  triggered as soon as its V column-half is projected; the exchange hides
  under V-half-1 projection + KtV chunk 1 + the whole Q phase.
- KtV chunk fetches issue after all weight/table DMAs (the SP engine runs
  its queue in order; an AR-dependent fetch would head-block weight loads).
- Weights and x are bf16 instead of f32r: same PE throughput, half DMA.
- PE warm-up trimmed 40 -> 8 junk matmuls (HAM activity budget).

Tricks kept from baseline:
- RoPE channel permutation: W_in's Q/K columns are pre-permuted (even
  channels then odd channels) on the host so the interleaved-pair rotation
  becomes contiguous-block arithmetic; the permutation cancels inside Q K^T.
- biases land via a pre-broadcast tensor add on the PSUM-evacuation pass
  (K/V) or an ACT Identity-activation per-partition bias (Q, transposed).
- group norm: stats from PSUM, then ygn = y*rstd - mean*rstd.
"""

from contextlib import ExitStack

import ml_dtypes
import numpy as np
import concourse.bass as bass
import concourse.tile as tile
from concourse import bacc, mybir, bass_utils

B, T, D = 4, 2048, 1024
TL = T // 2          # rows per core (sequence half)
EPS = 1e-5
THETA = 10000.0
NGROUP = 16          # groups per 512-column tile
GSIZE = 32

F32 = mybir.dt.float32
F32R = mybir.dt.float32r
BF16 = mybir.dt.bfloat16

KV_DT = BF16          # dtype of K/V activations + KtV collective
RG = [[0, 1], [2, 3], [4, 5], [6, 7]]
RELU = mybir.ActivationFunctionType.Relu
IDENT = mybir.ActivationFunctionType.Identity

_CACHE = {}


def _build(gn_trivial):
    nc = bacc.Bacc("TRN2", target_bir_lowering=False, debug=False, num_devices=8)

    xT = nc.dram_tensor("xT", [D, TL], BF16, kind="ExternalInput")
    wq = nc.dram_tensor("wq", [D, D], BF16, kind="ExternalInput")
    wkv = nc.dram_tensor("wkv", [4 * D, 512], BF16, kind="ExternalInput")
    bq = nc.dram_tensor("bq", [128, 8], F32, kind="ExternalInput")
    bkvb = nc.dram_tensor("bkvb", [128, 2 * D], F32, kind="ExternalInput")
    cosq = nc.dram_tensor("cosq", [D // 2, TL], BF16, kind="ExternalInput")
    sinq = nc.dram_tensor("sinq", [D // 2, TL], BF16, kind="ExternalInput")
    cosk = nc.dram_tensor("cosk", [TL, D // 2], BF16, kind="ExternalInput")
    sink = nc.dram_tensor("sink", [TL, D // 2], BF16, kind="ExternalInput")
    if not gn_trivial:
        gnw = nc.dram_tensor("gnw", [128, D], F32, kind="ExternalInput")
        gnb = nc.dram_tensor("gnb", [128, D], F32, kind="ExternalInput")
    out = nc.dram_tensor("out", [2 * TL, 512], BF16, kind="ExternalOutput")

    with tile.TileContext(nc) as tc, ExitStack() as st:
        psmall = st.enter_context(tc.tile_pool(name="small", bufs=1))
        pq = st.enter_context(tc.tile_pool(name="pq", bufs=1))
        pktv = st.enter_context(tc.tile_pool(name="pktv", bufs=1))
        pwq = st.enter_context(tc.tile_pool(name="pwq", bufs=1))
        ptabq = st.enter_context(tc.tile_pool(name="ptabq", bufs=1))
        pdram = st.enter_context(tc.tile_pool(name="pdram", bufs=1,
                                              space="DRAM"))

        bq_sb = psmall.tile([128, 8], F32, name="bq_sb")
        nc.sync.dma_start(bq_sb[:], bq[:])
        bkvb_sb = psmall.tile([128, 2 * D], F32, name="bkvb_sb")
        nc.sync.dma_start(bkvb_sb[:], bkvb[:])
        if not gn_trivial:
            gnw_sb = psmall.tile([128, D], F32, name="gnw_sb")
            nc.sync.dma_start(gnw_sb[:], gnw[:])
            gnb_sb = psmall.tile([128, D], F32, name="gnb_sb")
            nc.sync.dma_start(gnb_sb[:], gnb[:])
        eps_sb = psmall.tile([128, 1], F32, name="eps_sb")
        nc.vector.memset(eps_sb[:], EPS)

        qr = [pq.tile([128, TL], KV_DT, name=f"qr{j}", tag=f"qr{j}")
              for j in range(8)]
        ktv = [pktv.tile([128, D], KV_DT, name=f"ktv{c}", tag=f"ktv{c}")
               for c in range(8)]
        wq_sb = [pwq.tile([128, D], BF16, name=f"wq{d}", tag=f"wq{d}")
                 for d in range(8)]
        tabq = [(ptabq.tile([128, TL], BF16, name=f"cq{j}", tag=f"cq{j}"),
                 ptabq.tile([128, TL], BF16, name=f"sq{j}", tag=f"sq{j}"))
                for j in range(4)]
        cc_out = [pdram.tile([D, 512], KV_DT, name=f"cco{dh}", tag=f"cco{dh}")
                  for dh in range(2)]

        with tc.tile_pool(name="pxt", bufs=1) as pxt, \
             tc.tile_pool(name="ppsA", bufs=8, space="PSUM") as ppsA:
            xt = [pxt.tile([128, TL], BF16, name=f"xt{d}", tag=f"xt{d}")
                  for d in range(8)]

            # ========= phase A1: K'/V proj, K rope, chunked KtV + collectives
            with tc.tile_pool(name="pkv", bufs=1) as pkv, \
                 tc.tile_pool(name="pwkv", bufs=16) as pwkv, \
                 tc.tile_pool(name="pkm", bufs=1) as pkm, \
                 tc.tile_pool(name="ptabk", bufs=8) as ptabk, \
                 tc.tile_pool(name="ptmpk", bufs=2) as ptmpk, \
                 tc.tile_pool(name="ppart", bufs=6) as ppart:

                kr = [pkv.tile([128, D], KV_DT, name=f"kr{t}", tag=f"kr{t}")
                      for t in range(8)]
                v_sb = [pkv.tile([128, D], KV_DT, name=f"v{t}", tag=f"v{t}")
                        for t in range(8)]
                km = [pkm.tile([128, D], KV_DT, name=f"km{t}", tag=f"km{t}")
                      for t in range(8)]

                def kv_proj(cb, tbs, wv):
                    for tb in tbs:
                        ps = ppsA.tile([128, 512], F32, name=f"pskv{cb}_{tb}",
                                       tag="psA")
                        for d in range(8):
                            nc.tensor.matmul(ps[:],
                                             xt[d][:, tb * 128:(tb + 1) * 128],
                                             wv[d][:], start=(d == 0),
                                             stop=(d == 7))
                        bias_s = bkvb_sb[:, cb * 512:(cb + 1) * 512]
                        if cb < 2:   # K' half -> km (+bias; rope next)
                            nc.vector.tensor_add(
                                km[tb][:, cb * 512:(cb + 1) * 512], ps[:],
                                bias_s)
                        else:        # V half -> +bias then relu in place
                            vslc = v_sb[tb][:, (cb - 2) * 512:(cb - 1) * 512]
                            nc.vector.tensor_add(vslc, ps[:], bias_s)
                            nc.scalar.activation(vslc, vslc, RELU)

                def load_wv(cb):
                    wv = [pwkv.tile([128, 512], BF16, name=f"wv{cb}_{d}",
                                    tag="wv") for d in range(8)]
                    for d in range(8):
                        if cb == 0:
                            nc.sync.dma_start(xt[d][:],
                                              xT[d * 128:(d + 1) * 128, :])
                        nc.sync.dma_start(
                            wv[d][:], wkv[cb * D + d * 128:
                                          cb * D + (d + 1) * 128, :])
                    return wv

                def ktv_chunk(dh):
                    """KtV partial for D-column half dh + its pair AllReduce.

                    The result lands in cc_out[dh]; the SBUF fetch is issued
                    later (after all weight/table DMAs) to avoid head-blocking
                    the SP DMA queue on the collective.
                    """
                    cc_in = pdram.tile([D, 512], KV_DT, name=f"cci{dh}",
                                       tag=f"cci{dh}")
                    for d1c in range(8):
                        ps = ppsA.tile([128, 512], F32,
                                       name=f"psk2_{dh}_{d1c}", tag="psA")
                        for tb in range(8):
                            nc.tensor.matmul(
                                ps[:],
                                kr[tb][:, d1c * 128:(d1c + 1) * 128],
                                v_sb[tb][:, dh * 512:(dh + 1) * 512],
                                start=(tb == 0), stop=(tb == 7))
                        part = ppart.tile([128, 512], KV_DT,
                                          name=f"part{dh}_{d1c}", tag="part")
                        if d1c % 2 == 0:
                            nc.vector.tensor_copy(part[:], ps[:])
                        else:
                            nc.scalar.copy(part[:], ps[:])
                        nc.sync.dma_start(cc_in[d1c * 128:(d1c + 1) * 128, :],
                                          part[:])
                    nc.gpsimd.collective_compute(
                        "AllReduce", mybir.AluOpType.add,
                        ins=[cc_in[:].opt()], outs=[cc_out[dh][:].opt()],
                        replica_groups=RG)

                # PE warm-up during the initial DMA fill keeps the HAM
                # activity monitor at full clock; kept short (fake activity
                # costs real throttle budget later)
                warm = ptmpk.tile([128, 512], F32R, name="warm", tag="warm",
                                  bufs=1)
                nc.vector.memset(warm[:].bitcast(F32), 0.0)
                wps = ppsA.tile([128, 512], F32, name="wps", tag="psA")
                for i in range(8):
                    nc.tensor.matmul(wps[:], warm[:, 0:128], warm[:],
                                     start=(i == 0), stop=(i == 7))

                # K' projection (cols 0:1024 of [K'|V]) then rope per row-block
                wv01 = [load_wv(0), load_wv(1)]
                ktab = []
                for tb in range(8):
                    ck = ptabk.tile([128, 512], BF16, name=f"ck{tb}", tag="ck")
                    sk = ptabk.tile([128, 512], BF16, name=f"sk{tb}", tag="sk")
                    nc.sync.dma_start(ck[:], cosk[tb * 128:(tb + 1) * 128, :])
                    nc.sync.dma_start(sk[:], sink[tb * 128:(tb + 1) * 128, :])
                    ktab.append((ck, sk))
                kv_proj(0, range(8), wv01[0])
                # wv2/wv3 DMAs issue early (prefetch during the K projection;
                # the 16-buf wv pool lets them land while wv0/wv1 retire)
                wv2 = load_wv(2)
                kv_proj(1, range(8), wv01[1])
                wv3 = load_wv(3)
                for tb in range(8):
                    ck, sk = ktab[tb]
                    x1 = km[tb][:, 0:512]
                    x2 = km[tb][:, 512:1024]
                    t1 = ptmpk.tile([128, 512], KV_DT, name=f"t1k{tb}",
                                    tag="t1")
                    t2 = ptmpk.tile([128, 512], KV_DT, name=f"t2k{tb}",
                                    tag="t2")
                    nc.vector.tensor_mul(t1[:], x1, sk[:])       # x1*sin
                    nc.vector.tensor_mul(x1, x1, ck[:])          # x1 = x1*cos
                    nc.vector.tensor_mul(t2[:], x2, sk[:])       # x2*sin
                    nc.vector.tensor_sub(x1, x1, t2[:])          # r1
                    nc.vector.tensor_mul(x2, x2, ck[:])          # x2 = x2*cos
                    nc.vector.tensor_add(x2, x2, t1[:])          # r2
                    nc.scalar.activation(kr[tb][:, 0:512], x1, RELU)
                    nc.scalar.activation(kr[tb][:, 512:1024], x2, RELU)

                # V half 0 -> KtV chunk 0 + its collective, then V half 1 ->
                # chunk 1; each collective overlaps the following PE work.
                # wq + Q-table DMAs issue here: there is DMA slack, and A2
                # must not wait on them.
                kv_proj(2, range(8), wv2)
                ktv_chunk(0)
                for d in range(8):
                    nc.sync.dma_start(wq_sb[d][:],
                                      wq[d * 128:(d + 1) * 128, :])
                for j in range(4):
                    cq, sq = tabq[j]
                    nc.sync.dma_start(cq[:], cosq[j * 128:(j + 1) * 128, :])
                    nc.sync.dma_start(sq[:], sinq[j * 128:(j + 1) * 128, :])
                kv_proj(3, range(8), wv3)
                ktv_chunk(1)

            # ========= phase A2: Q' proj (transposed layout) + rope =========
            with tc.tile_pool(name="pqm", bufs=1) as pqm, \
                 tc.tile_pool(name="ptmpq", bufs=2) as ptmpq:

                # fetch the reduced KtV chunks straight into the y operand;
                # chunk 0 (the th=0 half) first so the y phase starts sooner.
                # All load-bearing DMAs are already issued, so blocking the
                # SP queue on the collectives is harmless.
                for dh in range(2):
                    for d1c in range(8):
                        nc.sync.dma_start(
                            ktv[d1c][:, dh * 512:(dh + 1) * 512],
                            cc_out[dh][d1c * 128:(d1c + 1) * 128, :])

                qm = [pqm.tile([128, TL], BF16, name=f"qm{j}", tag=f"qm{j}")
                      for j in range(8)]

                def q_proj(cp):
                    for th in range(2):
                        ps = ppsA.tile([128, 512], F32, name=f"psq{cp}_{th}",
                                       tag="psA")
                        for d in range(8):
                            nc.tensor.matmul(
                                ps[:], wq_sb[d][:, cp * 128:(cp + 1) * 128],
                                xt[d][:, th * 512:(th + 1) * 512],
                                start=(d == 0), stop=(d == 7))
                        nc.scalar.activation(
                            qm[cp][:, th * 512:(th + 1) * 512], ps[:],
                            IDENT, bias=bq_sb[:, cp:cp + 1])

                # pair (j, j+4) projected together so rope(j) starts while the
                # next pair is still on the PE
                for j in range(4):
                    q_proj(j)
                    q_proj(j + 4)
                    cq, sq = tabq[j]
                    x1 = qm[j][:]
                    x2 = qm[j + 4][:]
                    t1 = ptmpq.tile([128, TL], BF16, name=f"t1q{j}", tag="t1",
                                    bufs=1)
                    t2 = ptmpq.tile([128, TL], BF16, name=f"t2q{j}", tag="t2",
                                    bufs=1)
                    nc.gpsimd.tensor_mul(t1[:], x1, sq[:])       # x1*sin
                    nc.vector.tensor_mul(x1, x1, cq[:])          # x1*cos
                    nc.vector.tensor_mul(t2[:], x2, sq[:])       # x2*sin
                    nc.vector.tensor_sub(x1, x1, t2[:])          # r1
                    nc.vector.tensor_mul(x2, x2, cq[:])          # x2*cos
                    nc.vector.tensor_add(x2, x2, t1[:])          # r2
                    nc.scalar.activation(qr[j][:], x1, RELU)
                    nc.scalar.activation(qr[j + 4][:], x2, RELU)

        # ================= phase C: y = Q' @ KtV + fused group norm ========
        # row-blocks processed in PAIRS: one [128,1024] PSUM tile spans two
        # banks, halving the per-op overhead of the stat chain. Stats:
        #   varX = sum(y^2) - mean*sum(y);  rstd = 1/sqrt(varX/32 + eps)
        # (the /32 and +eps fold into the Sqrt activation's scale/bias).
        # Work is spread over scalar/vector/gpsimd so no engine backlogs.
        with tc.tile_pool(name="pgn", bufs=3) as pgn, \
             tc.tile_pool(name="pstat", bufs=4) as pstat, \
             tc.tile_pool(name="ppsY", bufs=1, space="PSUM") as ppsY:
            inv32 = 1.0 / GSIZE
            NG2 = 2 * NGROUP
            for th in range(2):
                for pb in range(4):
                    ps = ppsY.tile([128, 1024], F32, name=f"psy{th}_{pb}",
                                   tag=f"psy{pb}")
                    for half in range(2):
                        tb = 2 * pb + half
                        dst = ps[:, half * 512:(half + 1) * 512]
                        for c in range(8):
                            nc.tensor.matmul(
                                dst, qr[c][:, tb * 128:(tb + 1) * 128],
                                ktv[c][:, th * 512:(th + 1) * 512],
                                start=(c == 0), stop=(c == 7))
                    ps4 = ps[:].rearrange("p (u g c) -> p u g c", u=2,
                                          g=NGROUP)
                    sums = pstat.tile([128, NG2], F32, name=f"su{th}_{pb}",
                                      tag="su")
                    su3 = sums[:].rearrange("p (u g) -> p u g", u=2)
                    nc.vector.reduce_sum(su3, ps4, axis=mybir.AxisListType.X)
                    sqt = pgn.tile([128, 1024], F32, name=f"sqt{th}_{pb}",
                                   tag="sqt")
                    nc.scalar.square(sqt[:], ps[:])
                    sums2 = pstat.tile([128, NG2], F32, name=f"s2{th}_{pb}",
                                       tag="s2")
                    nc.vector.reduce_sum(
                        sums2[:].rearrange("p (u g) -> p u g", u=2),
                        sqt[:].rearrange("p (u g c) -> p u g c", u=2,
                                         g=NGROUP),
                        axis=mybir.AxisListType.X)
                    mean = pstat.tile([128, NG2], F32, name=f"mn{th}_{pb}",
                                      tag="mn")
                    nc.scalar.activation(mean[:], sums[:], IDENT, scale=inv32)
                    msu = pstat.tile([128, NG2], F32, name=f"mq{th}_{pb}",
                                     tag="mq")
                    nc.gpsimd.tensor_mul(msu[:], mean[:], sums[:])
                    varx = pstat.tile([128, NG2], F32, name=f"va{th}_{pb}",
                                      tag="va")
                    nc.gpsimd.tensor_sub(varx[:], sums2[:], msu[:])
                    sd = pstat.tile([128, NG2], F32, name=f"sd{th}_{pb}",
                                    tag="sd")
                    nc.scalar.activation(sd[:], varx[:],
                                         mybir.ActivationFunctionType.Sqrt,
                                         scale=inv32, bias=eps_sb[:])
                    rstd = pstat.tile([128, NG2], F32, name=f"rs{th}_{pb}",
                                      tag="rs")
                    nc.vector.reciprocal(rstd[:], sd[:])
                    mr = pstat.tile([128, NG2], F32, name=f"mr{th}_{pb}",
                                    tag="mr")
                    nc.gpsimd.tensor_mul(mr[:], mean[:], rstd[:])

                    # y*rstd stays f32: its magnitude is ~mean/std (large),
                    # so a bf16 rounding BEFORE the mean subtraction would be
                    # amplified relative to the normalized output. Only the
                    # final subtract emits bf16.
                    r3 = rstd[:].rearrange("p (u g) -> p u g", u=2)
                    m3 = mr[:].rearrange("p (u g) -> p u g", u=2)
                    ygn = pgn.tile([128, 1024], F32, name=f"yg{th}_{pb}",
                                   tag="ygn")
                    y4 = ygn[:].rearrange("p (u g c) -> p u g c", u=2,
                                          g=NGROUP)
                    nc.vector.tensor_mul(
                        y4, ps4, r3.broadcast_to([128, 2, NGROUP, GSIZE]))
                    yout = pgn.tile([128, 1024], KV_DT, name=f"yo{th}_{pb}",
                                    tag="yout")
                    yo4 = yout[:].rearrange("p (u g c) -> p u g c", u=2,
                                            g=NGROUP)
                    cs = slice(th * 512, (th + 1) * 512)
                    if gn_trivial:
                        nc.gpsimd.tensor_sub(
                            yo4, y4, m3.broadcast_to([128, 2, NGROUP, GSIZE]))
                    else:
                        nc.gpsimd.tensor_sub(
                            y4, y4, m3.broadcast_to([128, 2, NGROUP, GSIZE]))
                        for half in range(2):
                            hs = slice(half * 512, (half + 1) * 512)
                            nc.gpsimd.tensor_mul(ygn[:, hs], ygn[:, hs],
                                                 gnw_sb[:, cs])
                            nc.gpsimd.tensor_add(yout[:, hs], ygn[:, hs],
                                                 gnb_sb[:, cs])
                    for half in range(2):
                        tb = 2 * pb + half
                        ro = th * TL + tb * 128
                        nc.sync.dma_start(
                            out[ro:ro + 128, :],
                            yout[:, half * 512:(half + 1) * 512])

    nc.compile()
    return nc


def _get_nc(gn_trivial):
    key = ("nc", gn_trivial)
    if key not in _CACHE:
        _CACHE[key] = _build(gn_trivial)
    return _CACHE[key]


def _make_in_maps(x, W_in, b_in, gn_weight, gn_bias, gn_trivial):
    perm = np.concatenate([np.arange(0, D, 2), np.arange(1, D, 2)])
    wq_h = np.ascontiguousarray(W_in[:, 0:D][:, perm]).astype(
        ml_dtypes.bfloat16)
    wk = W_in[:, D:2 * D][:, perm]
    wv = W_in[:, 2 * D:3 * D]
    wkv_cat = np.concatenate([wk, wv], axis=1)          # [D, 2D]
    wkv_h = np.ascontiguousarray(
        np.concatenate([wkv_cat[:, cb * 512:(cb + 1) * 512]
                        for cb in range(4)], axis=0)).astype(
        ml_dtypes.bfloat16)                              # [4D, 512]
    bq_h = np.ascontiguousarray(b_in[0:D][perm].reshape(8, 128).T)
    bkv_row = np.concatenate([b_in[D:2 * D][perm], b_in[2 * D:3 * D]])
    bkvb_h = np.ascontiguousarray(
        np.broadcast_to(bkv_row[None, :], (128, 2 * D))).astype(np.float32)
    gnw_h = np.ascontiguousarray(
        np.broadcast_to(np.float32(gn_weight)[None, :], (128, D)))
    gnb_h = np.ascontiguousarray(
        np.broadcast_to(np.float32(gn_bias)[None, :], (128, D)))

    inv_freq = (1.0 / (THETA ** (np.arange(0, D, 2, dtype=np.float32) / D))
                ).astype(np.float32)

    in_maps = []
    for core in range(8):
        b, h = divmod(core, 2)
        ts = np.arange(h * TL, (h + 1) * TL, dtype=np.float32)
        freqs = ts[:, None] * inv_freq[None, :]      # [TL, 512]
        cos_n = np.cos(freqs).astype(np.float32)
        sin_n = np.sin(freqs).astype(np.float32)
        xT_h = np.ascontiguousarray(
            x[b, h * TL:(h + 1) * TL, :].T).astype(ml_dtypes.bfloat16)
        m = {
            "xT": xT_h, "wq": wq_h, "wkv": wkv_h, "bq": bq_h, "bkvb": bkvb_h,
            "cosq": np.ascontiguousarray(cos_n.T).astype(ml_dtypes.bfloat16),
            "sinq": np.ascontiguousarray(sin_n.T).astype(ml_dtypes.bfloat16),
            "cosk": cos_n.astype(ml_dtypes.bfloat16),
            "sink": sin_n.astype(ml_dtypes.bfloat16),
        }
        if not gn_trivial:
            m["gnw"] = gnw_h
            m["gnb"] = gnb_h
        in_maps.append(m)
    return in_maps


def kernel(x, W_in, b_in, gn_weight, gn_bias, _trace=False):
    x = np.asarray(x, dtype=np.float32)
    W_in = np.asarray(W_in, dtype=np.float32)
    b_in = np.asarray(b_in, dtype=np.float32)
    gn_weight = np.asarray(gn_weight, dtype=np.float32)
    gn_bias = np.asarray(gn_bias, dtype=np.float32)

    gn_trivial = bool(np.all(gn_weight == 1.0) and np.all(gn_bias == 0.0))
    nc = _get_nc(gn_trivial)
    in_maps = _make_in_maps(x, W_in, b_in, gn_weight, gn_bias, gn_trivial)
    res = bass_utils.run_bass_kernel_spmd(nc, in_maps, core_ids=list(range(8)),
                                          trace=_trace)
    _CACHE["last_result"] = res

    outs = [np.concatenate([np.float32(res.results[i]["out"][:TL]),
                            np.float32(res.results[i]["out"][TL:])], axis=1)
            for i in range(8)]                            # [TL, D] each
    full = np.empty((B, T, D), dtype=np.float32)
    for b in range(B):
        y_gn = np.concatenate([outs[2 * b], outs[2 * b + 1]], axis=0)  # [T,D]
        full[b] = y_gn.T.reshape(T, D)
    return full
